# revision 8
# baseline (speedup 1.0000x reference)
"""L1-distance attention (NeuralDictionary) over 8 TRN2 NeuronCores.

Full inputs: query [1024] f32, keys [65536, 1024] f32, values [65536, 1024] f32.
Output: softmax(-sum|keys - q|) @ values -> [1024] f32.

Sharding: keys/values split along capacity across 8 cores (8192 rows each).
Each core computes, per group of keys, an online-softmax partial triple
(m_g = max score, per-partition exp sums, unnormalized weighted values); the
host combines the per-core, per-group triples exactly (flash-attention-style)
in fp64. Group sizes shrink toward the end so almost no compute remains after
the last DMA byte lands.
"""

import numpy as np

P = 128            # SBUF partitions
D = 1024           # feature dim
ROWS = 8192        # keys per core (65536 / 8)
CHUNKS = ROWS // P # 64 chunks of 128 keys
BC = 4             # chunks per DMA block
NBLK = CHUNKS // BC
GROUPS = [8] * 7 + [4, 2, 1, 1]   # chunks per softmax group (sum = 64)
NG = len(GROUPS)
NCORES = 8

assert sum(GROUPS) == CHUNKS

_nc_cache = None


def _patch_tile_drain():
    """Stock TileContext._drain_and_barrier piles every outstanding proc-sem
    wait onto a single InstDrain, but walrus codegen caps non-ESEM
    instructions at 1 sync wait ("Too many sync wait commands"). Re-emit the
    waits as individual wait instructions on the same (SP) queue instead."""
    import bass_rust as _br
    import concourse.tile as tile
    from concourse.vector_clock import ScopedClock

    if getattr(tile.TileContext, "_drain_split_patch", False):
        return

    def _split_drain_and_barrier(self, tick_clock, wait_clock):
        nc = self.nc
        probe = nc.sync.drain()
        wait_clock.add_sem_waits(
            probe.ins, ScopedClock({None: tick_clock.global_clock})
        )
        waits = list(probe.ins.sync_info.on_wait)
        if len(waits) > 1:
            probe.ins.sync_info = _br.SyncInfo(on_wait=[], on_update=[])
            by_num = {h.num: h for h in self.sems.allocated().values()}
            for w in waits:
                nc.sync.wait_ge(by_num[w.id], w.wait_value)
        nc.all_engine_barrier()
        popped = nc._tile_sem_poison_stack.pop()
        assert popped is self._sem_poison
        nc.clear_and_free_semaphores(list(self.sems.allocated().values()))
        nc.all_engine_barrier()

    tile.TileContext._drain_and_barrier = _split_drain_and_barrier
    tile.TileContext._drain_split_patch = True


def _legalize_waits(nc):
    """This walrus build encodes at most 1 sync wait on a regular instruction
    (2 on an EventSemaphore); Tile attaches up to 3. Spill excess waits onto
    EventSemaphore instructions inserted right before, on the same engine."""
    import bass_rust as _br
    import concourse.mybir as mybir

    n_new = 0
    for f in nc.m.functions:
        for bb in f.blocks:
            insts = bb.instructions
            i = 0
            while i < len(insts):
                inst = insts[i]
                si = inst.sync_info
                cap = 2 if isinstance(inst, mybir.InstEventSemaphore) else 1
                if si is not None and len(si.on_wait) > cap:
                    waits = list(si.on_wait)
                    keep, spill = waits[:cap], waits[cap:]
                    inst.sync_info = _br.SyncInfo(
                        on_wait=keep, on_update=list(si.on_update)
                    )
                    pos = i
                    for j in range(0, len(spill), 2):
                        e = mybir.InstEventSemaphore(
                            name=f"esem-split-{n_new}", ins=[], outs=[]
                        )
                        n_new += 1
                        e.engine = inst.engine
                        e.sync_info = _br.SyncInfo(
                            on_wait=spill[j : j + 2], on_update=[]
                        )
                        insts.insert(pos, e)
                        pos += 1
                        i += 1
                i += 1
    return n_new


def build_nc(legalize=True):
    """Build the per-core Bass program (same SPMD program on all 8 cores).

    legalize=False skips the walrus wait-encoding fixup (CoreSim chokes on the
    injected EventSemaphore instructions; semantics are identical)."""
    from contextlib import ExitStack

    import concourse.bass as bass
    import concourse.mybir as mybir
    import concourse.tile as tile
    from concourse.masks import make_identity

    _patch_tile_drain()

    f32 = mybir.dt.float32
    f32r = mybir.dt.float32r  # fp32 bytes, fast reduced-precision matmul path
    AX = mybir.AxisListType.X
    OP = mybir.AluOpType
    AF = mybir.ActivationFunctionType

    nc = bass.Bass("TRN2", target_bir_lowering=False, debug=False, num_devices=1)
    q = nc.dram_tensor("q", [P, D], f32, kind="ExternalInput").ap()
    keys = nc.dram_tensor("keys", [ROWS, D], f32, kind="ExternalInput").ap()
    values = nc.dram_tensor("values", [ROWS, D], f32r, kind="ExternalInput").ap()
    vout = nc.dram_tensor("vout", [1, NG * D], f32, kind="ExternalOutput").ap()
    esums = nc.dram_tensor("esums", [P, NG], f32, kind="ExternalOutput").ap()
    gmin = nc.dram_tensor("gmin", [1, NG], f32, kind="ExternalOutput").ap()

    with tile.TileContext(nc) as tc, ExitStack() as ctx:
        const_pool = ctx.enter_context(tc.tile_pool(name="const", bufs=1))
        kt_pool = ctx.enter_context(tc.tile_pool(name="kt", bufs=5))
        vt_pool = ctx.enter_context(tc.tile_pool(name="vt", bufs=4))
        d_pool = ctx.enter_context(tc.tile_pool(name="dtile", bufs=3))
        small = ctx.enter_context(tc.tile_pool(name="small", bufs=2))
        wsum_pool = ctx.enter_context(tc.tile_pool(name="wsum", bufs=2))
        vo_pool = ctx.enter_context(tc.tile_pool(name="vo", bufs=2))
        out_pool = ctx.enter_context(tc.tile_pool(name="outp", bufs=1))
        psum_v = ctx.enter_context(tc.tile_pool(name="psv", bufs=2, space="PSUM"))
        psum_s = ctx.enter_context(tc.tile_pool(name="pss", bufs=2, space="PSUM"))

        kts = {}
        vts = {}

        def get_kt(b):
            if b not in kts:
                t = kt_pool.tile([P, BC, D], f32, tag="kt")
                r0 = b * BC * P
                nc.sync.dma_start(
                    t[:],
                    keys[r0 : r0 + BC * P, :].rearrange("(k p) c -> p k c", p=P),
                )
                kts[b] = t
            return kts[b]

        def get_vt(b):
            if b not in vts:
                t = vt_pool.tile([P, BC, D], f32r, tag="vt")
                r0 = b * BC * P
                # SWDGE (gpsimd) ring: keeps descriptor-gen off ACT/SP
                nc.gpsimd.dma_start(
                    t[:],
                    values[r0 : r0 + BC * P, :].rearrange("(k p) c -> p k c", p=P),
                )
            else:
                t = vts[b]
                return t
            vts[b] = t
            return t

        # kick off the first loads before the constant setup
        get_kt(0)
        get_vt(0)

        qb = const_pool.tile([P, D], f32)
        nc.sync.dma_start(qb[:], q)
        ident = const_pool.tile([P, P], f32)
        make_identity(nc, ident[:])
        ones = const_pool.tile([1, P], f32)
        nc.vector.memset(ones[:], 1.0)

        esums_sb = out_pool.tile([P, NG], f32)
        gmin_sb = out_pool.tile([1, NG], f32)

        j0 = 0  # first chunk of current group
        for gi, gs in enumerate(GROUPS):
            # Phase A: L1 scores. sums[p, k] = sum_c |keys[(j0+k)P+p, c] - q[c]|
            sums_g = wsum_pool.tile([P, gs], f32, tag="sums")
            for k in range(gs):
                j = j0 + k
                kt = get_kt(j // BC)
                get_vt(j // BC)  # prefetch values alongside keys
                dt_ = d_pool.tile([P, D], f32, tag="dt")
                nc.vector.tensor_sub(dt_[:], kt[:, j % BC, :], qb[:])
                nc.scalar.activation(
                    dt_[:], dt_[:], AF.Abs, accum_out=sums_g[:, k : k + 1]
                )

            # Group min of sums (= -max score) -> scalar, broadcast to [P,1]
            pmin = small.tile([P, 1], f32, tag="pmin")
            nc.vector.tensor_reduce(pmin[:], sums_g[:], AX, OP.min)
            pminT = psum_s.tile([1, P], f32, tag="pminT")
            nc.tensor.transpose(pminT[:], pmin[:], ident[:])
            nc.vector.tensor_reduce(gmin_sb[0:1, gi : gi + 1], pminT[:], AX, OP.min)
            bias_ps = psum_s.tile([P, 1], f32, tag="biasps")
            nc.tensor.matmul(bias_ps[:], ones[:], gmin_sb[0:1, gi : gi + 1])
            bias_sb = small.tile([P, 1], f32, tag="bias")
            nc.vector.tensor_copy(bias_sb[:], bias_ps[:])

            # w = exp(min_sums - sums); per-partition sums of w into esums col
            wg = wsum_pool.tile([P, gs], f32r, tag="wg")
            nc.scalar.activation(
                wg[:],
                sums_g[:],
                AF.Exp,
                bias=bias_sb[:],
                scale=-1.0,
                accum_out=esums_sb[:, gi : gi + 1],
            )

            # Phase B: v_g = w_g @ values_g  (fp32r: 1 cyc/row at N=512)
            pv0 = psum_v.tile([1, 512], f32, tag="pv0")
            pv1 = psum_v.tile([1, 512], f32, tag="pv1")
            for k in range(gs):
                j = j0 + k
                vt = get_vt(j // BC)
                nc.tensor.matmul(
                    pv0[:], wg[:, k : k + 1], vt[:, j % BC, 0:512],
                    start=(k == 0), stop=(k == gs - 1),
                )
                nc.tensor.matmul(
                    pv1[:], wg[:, k : k + 1], vt[:, j % BC, 512:1024],
                    start=(k == 0), stop=(k == gs - 1),
                )
            vo = vo_pool.tile([1, D], f32, tag="vo")
            nc.vector.tensor_copy(vo[0:1, 0:512], pv0[:])
            nc.vector.tensor_copy(vo[0:1, 512:1024], pv1[:])
            nc.sync.dma_start(vout[0:1, gi * D : (gi + 1) * D], vo[:])
            j0 += gs

        nc.sync.dma_start(esums, esums_sb[:])
        nc.sync.dma_start(gmin, gmin_sb[:])

    if legalize:
        _legalize_waits(nc)
    return nc


def _get_nc():
    global _nc_cache
    if _nc_cache is None:
        _nc_cache = build_nc()
    return _nc_cache


def make_in_maps(query, keys, values):
    query = np.asarray(query, dtype=np.float32)
    keys = np.asarray(keys, dtype=np.float32)
    values = np.asarray(values, dtype=np.float32)
    qb = np.ascontiguousarray(np.broadcast_to(query[None, :], (P, D)))
    in_maps = []
    for i in range(NCORES):
        in_maps.append(
            {
                "q": qb,
                "keys": np.ascontiguousarray(keys[i * ROWS : (i + 1) * ROWS]),
                "values": np.ascontiguousarray(values[i * ROWS : (i + 1) * ROWS]),
            }
        )
    return in_maps


def combine(results):
    """Online-softmax combine of the per-core, per-group partials (fp64)."""
    v = np.stack(
        [np.asarray(r["vout"]).reshape(NG, D) for r in results]
    ).astype(np.float64)                                   # [cores, NG, D]
    es = np.stack([np.asarray(r["esums"]) for r in results]).astype(np.float64)
    gm = np.stack(
        [np.asarray(r["gmin"]).reshape(NG) for r in results]
    ).astype(np.float64)                                   # [cores, NG] min sums
    m = -gm                                                # group max scores
    s = es.sum(axis=1)                                     # [cores, NG]
    M = m.max()
    alpha = np.exp(m - M)                                  # [cores, NG]
    num = (alpha[:, :, None] * v).sum(axis=(0, 1))         # [D]
    den = (alpha * s).sum()
    return (num / den).astype(np.float32)


def kernel(query, keys, values):
    from concourse.bass_utils import run_bass_kernel_spmd

    nc = _get_nc()
    in_maps = make_in_maps(query, keys, values)
    res = run_bass_kernel_spmd(nc, in_maps, core_ids=list(range(NCORES)))
    return combine(res.results)


# revision 9
# speedup vs baseline: 1.3437x; 1.3437x over previous
"""L1-distance attention (NeuralDictionary) over 8 TRN2 NeuronCores.

Full inputs: query [1024] f32, keys [65536, 1024] f32, values [65536, 1024] f32.
Output: softmax(-sum|keys - q|) @ values -> [1024] f32.

Sharding: keys/values split along capacity across 8 cores (8192 rows each).
Each core computes, per group of keys, an online-softmax partial triple
(m_g = max score, per-partition exp sums, unnormalized weighted values); the
host combines the per-core, per-group triples exactly (flash-attention-style)
in fp64. Group sizes shrink toward the end so almost no compute remains after
the last DMA byte lands.
"""

import numpy as np

P = 128            # SBUF partitions
D = 1024           # feature dim
ROWS = 8192        # keys per core (65536 / 8)
CHUNKS = ROWS // P # 64 chunks of 128 keys
BC = 4             # chunks per DMA block
NBLK = CHUNKS // BC
GROUPS = [8] * 7 + [4, 2, 1, 1]   # chunks per softmax group (sum = 64)
NG = len(GROUPS)
NCORES = 8

assert sum(GROUPS) == CHUNKS

_nc_cache = None


def _patch_tile_drain():
    """Stock TileContext._drain_and_barrier piles every outstanding proc-sem
    wait onto a single InstDrain, but walrus codegen caps non-ESEM
    instructions at 1 sync wait ("Too many sync wait commands"). Re-emit the
    waits as individual wait instructions on the same (SP) queue instead."""
    import bass_rust as _br
    import concourse.tile as tile
    from concourse.vector_clock import ScopedClock

    if getattr(tile.TileContext, "_drain_split_patch", False):
        return

    def _split_drain_and_barrier(self, tick_clock, wait_clock):
        nc = self.nc
        probe = nc.sync.drain()
        wait_clock.add_sem_waits(
            probe.ins, ScopedClock({None: tick_clock.global_clock})
        )
        waits = list(probe.ins.sync_info.on_wait)
        if len(waits) > 1:
            probe.ins.sync_info = _br.SyncInfo(on_wait=[], on_update=[])
            by_num = {h.num: h for h in self.sems.allocated().values()}
            for w in waits:
                nc.sync.wait_ge(by_num[w.id], w.wait_value)
        nc.all_engine_barrier()
        popped = nc._tile_sem_poison_stack.pop()
        assert popped is self._sem_poison
        nc.clear_and_free_semaphores(list(self.sems.allocated().values()))
        nc.all_engine_barrier()

    tile.TileContext._drain_and_barrier = _split_drain_and_barrier
    tile.TileContext._drain_split_patch = True


def _legalize_waits(nc):
    """This walrus build encodes at most 1 sync wait on a regular instruction
    (2 on an EventSemaphore); Tile attaches up to 3. Spill excess waits onto
    EventSemaphore instructions inserted right before, on the same engine."""
    import bass_rust as _br
    import concourse.mybir as mybir

    n_new = 0
    for f in nc.m.functions:
        for bb in f.blocks:
            insts = bb.instructions
            i = 0
            while i < len(insts):
                inst = insts[i]
                si = inst.sync_info
                cap = 2 if isinstance(inst, mybir.InstEventSemaphore) else 1
                if si is not None and len(si.on_wait) > cap:
                    waits = list(si.on_wait)
                    keep, spill = waits[:cap], waits[cap:]
                    inst.sync_info = _br.SyncInfo(
                        on_wait=keep, on_update=list(si.on_update)
                    )
                    pos = i
                    for j in range(0, len(spill), 2):
                        e = mybir.InstEventSemaphore(
                            name=f"esem-split-{n_new}", ins=[], outs=[]
                        )
                        n_new += 1
                        e.engine = inst.engine
                        e.sync_info = _br.SyncInfo(
                            on_wait=spill[j : j + 2], on_update=[]
                        )
                        insts.insert(pos, e)
                        pos += 1
                        i += 1
                i += 1
    return n_new


def build_nc(legalize=True):
    """Build the per-core Bass program (same SPMD program on all 8 cores).

    legalize=False skips the walrus wait-encoding fixup (CoreSim chokes on the
    injected EventSemaphore instructions; semantics are identical)."""
    from contextlib import ExitStack

    import concourse.bass as bass
    import concourse.mybir as mybir
    import concourse.tile as tile
    from concourse.masks import make_identity

    _patch_tile_drain()

    f32 = mybir.dt.float32
    f32r = mybir.dt.float32r  # fp32 bytes, fast reduced-precision matmul path
    AX = mybir.AxisListType.X
    OP = mybir.AluOpType
    AF = mybir.ActivationFunctionType

    nc = bass.Bass("TRN2", target_bir_lowering=False, debug=False, num_devices=1)
    q = nc.dram_tensor("q", [P, D], f32, kind="ExternalInput").ap()
    keys = nc.dram_tensor("keys", [ROWS, D], f32, kind="ExternalInput").ap()
    values = nc.dram_tensor("values", [ROWS, D], f32r, kind="ExternalInput").ap()
    vout = nc.dram_tensor("vout", [1, NG * D], f32, kind="ExternalOutput").ap()
    esums = nc.dram_tensor("esums", [P, NG], f32, kind="ExternalOutput").ap()
    gmin = nc.dram_tensor("gmin", [1, NG], f32, kind="ExternalOutput").ap()

    with tile.TileContext(nc) as tc, ExitStack() as ctx:
        const_pool = ctx.enter_context(tc.tile_pool(name="const", bufs=1))
        kt_pool = ctx.enter_context(tc.tile_pool(name="kt", bufs=5))
        vt_pool = ctx.enter_context(tc.tile_pool(name="vt", bufs=4))
        d_pool = ctx.enter_context(tc.tile_pool(name="dtile", bufs=3))
        small = ctx.enter_context(tc.tile_pool(name="small", bufs=2))
        wsum_pool = ctx.enter_context(tc.tile_pool(name="wsum", bufs=2))
        vo_pool = ctx.enter_context(tc.tile_pool(name="vo", bufs=2))
        out_pool = ctx.enter_context(tc.tile_pool(name="outp", bufs=1))
        psum_v = ctx.enter_context(tc.tile_pool(name="psv", bufs=2, space="PSUM"))
        psum_s = ctx.enter_context(tc.tile_pool(name="pss", bufs=2, space="PSUM"))

        kts = {}
        vts = {}

        def get_kt(b):
            if b not in kts:
                t = kt_pool.tile([P, BC, D], f32, tag="kt")
                r0 = b * BC * P
                nc.sync.dma_start(
                    t[:],
                    keys[r0 : r0 + BC * P, :].rearrange("(k p) c -> p k c", p=P),
                )
                kts[b] = t
            return kts[b]

        def get_vt(b):
            if b not in vts:
                t = vt_pool.tile([P, BC, D], f32r, tag="vt")
                r0 = b * BC * P
                # ACT HWDGE ring so keys/values transfers overlap
                nc.scalar.dma_start(
                    t[:],
                    values[r0 : r0 + BC * P, :].rearrange("(k p) c -> p k c", p=P),
                )
            else:
                t = vts[b]
                return t
            vts[b] = t
            return t

        # kick off the first loads before the constant setup
        get_kt(0)
        get_vt(0)

        qb = const_pool.tile([P, D], f32)
        nc.sync.dma_start(qb[:], q)
        ident = const_pool.tile([P, P], f32)
        make_identity(nc, ident[:])
        ones = const_pool.tile([1, P], f32)
        nc.vector.memset(ones[:], 1.0)

        esums_sb = out_pool.tile([P, NG], f32)
        gmin_sb = out_pool.tile([1, NG], f32)

        j0 = 0  # first chunk of current group
        for gi, gs in enumerate(GROUPS):
            # Phase A: L1 scores. sums[p, k] = sum_c |keys[(j0+k)P+p, c] - q[c]|
            sums_g = wsum_pool.tile([P, gs], f32, tag="sums")
            for k in range(gs):
                j = j0 + k
                kt = get_kt(j // BC)
                get_vt(j // BC)  # prefetch values alongside keys
                dt_ = d_pool.tile([P, D], f32, tag="dt")
                nc.vector.tensor_sub(dt_[:], kt[:, j % BC, :], qb[:])
                nc.scalar.activation(
                    dt_[:], dt_[:], AF.Abs, accum_out=sums_g[:, k : k + 1]
                )

            # Group min of sums (= -max score) -> scalar, broadcast to [P,1]
            pmin = small.tile([P, 1], f32, tag="pmin")
            nc.vector.tensor_reduce(pmin[:], sums_g[:], AX, OP.min)
            pminT = psum_s.tile([1, P], f32, tag="pminT")
            nc.tensor.transpose(pminT[:], pmin[:], ident[:])
            nc.vector.tensor_reduce(gmin_sb[0:1, gi : gi + 1], pminT[:], AX, OP.min)
            bias_ps = psum_s.tile([P, 1], f32, tag="biasps")
            nc.tensor.matmul(bias_ps[:], ones[:], gmin_sb[0:1, gi : gi + 1])
            bias_sb = small.tile([P, 1], f32, tag="bias")
            nc.vector.tensor_copy(bias_sb[:], bias_ps[:])

            # w = exp(min_sums - sums); per-partition sums of w into esums col
            wg = wsum_pool.tile([P, gs], f32r, tag="wg")
            nc.scalar.activation(
                wg[:],
                sums_g[:],
                AF.Exp,
                bias=bias_sb[:],
                scale=-1.0,
                accum_out=esums_sb[:, gi : gi + 1],
            )

            # Phase B: v_g = w_g @ values_g  (fp32r: 1 cyc/row at N=512)
            pv0 = psum_v.tile([1, 512], f32, tag="pv0")
            pv1 = psum_v.tile([1, 512], f32, tag="pv1")
            for k in range(gs):
                j = j0 + k
                vt = get_vt(j // BC)
                nc.tensor.matmul(
                    pv0[:], wg[:, k : k + 1], vt[:, j % BC, 0:512],
                    start=(k == 0), stop=(k == gs - 1),
                )
                nc.tensor.matmul(
                    pv1[:], wg[:, k : k + 1], vt[:, j % BC, 512:1024],
                    start=(k == 0), stop=(k == gs - 1),
                )
            vo = vo_pool.tile([1, D], f32, tag="vo")
            nc.vector.tensor_copy(vo[0:1, 0:512], pv0[:])
            nc.vector.tensor_copy(vo[0:1, 512:1024], pv1[:])
            nc.sync.dma_start(vout[0:1, gi * D : (gi + 1) * D], vo[:])
            j0 += gs

        nc.sync.dma_start(esums, esums_sb[:])
        nc.sync.dma_start(gmin, gmin_sb[:])

    if legalize:
        _legalize_waits(nc)
    return nc


def _get_nc():
    global _nc_cache
    if _nc_cache is None:
        _nc_cache = build_nc()
    return _nc_cache


def make_in_maps(query, keys, values):
    query = np.asarray(query, dtype=np.float32)
    keys = np.asarray(keys, dtype=np.float32)
    values = np.asarray(values, dtype=np.float32)
    qb = np.ascontiguousarray(np.broadcast_to(query[None, :], (P, D)))
    in_maps = []
    for i in range(NCORES):
        in_maps.append(
            {
                "q": qb,
                "keys": np.ascontiguousarray(keys[i * ROWS : (i + 1) * ROWS]),
                "values": np.ascontiguousarray(values[i * ROWS : (i + 1) * ROWS]),
            }
        )
    return in_maps


def combine(results):
    """Online-softmax combine of the per-core, per-group partials (fp64)."""
    v = np.stack(
        [np.asarray(r["vout"]).reshape(NG, D) for r in results]
    ).astype(np.float64)                                   # [cores, NG, D]
    es = np.stack([np.asarray(r["esums"]) for r in results]).astype(np.float64)
    gm = np.stack(
        [np.asarray(r["gmin"]).reshape(NG) for r in results]
    ).astype(np.float64)                                   # [cores, NG] min sums
    m = -gm                                                # group max scores
    s = es.sum(axis=1)                                     # [cores, NG]
    M = m.max()
    alpha = np.exp(m - M)                                  # [cores, NG]
    num = (alpha[:, :, None] * v).sum(axis=(0, 1))         # [D]
    den = (alpha * s).sum()
    return (num / den).astype(np.float32)


def kernel(query, keys, values):
    from concourse.bass_utils import run_bass_kernel_spmd

    nc = _get_nc()
    in_maps = make_in_maps(query, keys, values)
    res = run_bass_kernel_spmd(nc, in_maps, core_ids=list(range(NCORES)))
    return combine(res.results)
